# revision 12
# baseline (speedup 1.0000x reference)
"""Trainium2 Bass kernel for EnhancedMambaMixer (B=2, L=1024, H=1024, D=2048, N=16, K=4, R=64).

Sharding: 8-way tensor-parallel over intermediate_size D (256 channels/core).
Each core computes its D-shard of in_proj/conv/scan and a partial out_proj;
a 786KB in-kernel AllReduce combines the x_proj partials (dt_lr/B/C are
reductions over the full D). Host sums the 8 out_proj partials.

Layout on-chip: channels in partitions, time fused as B*L=2048 in the free dim.
"""

import ml_dtypes
import numpy as np

# Problem constants (hardcoded; kernel.py must be self-contained).
B, L, H = 2, 1024, 1024
D = 2048
N = 16
K = 4
R = 64
NCORES = 8
DP = D // NCORES          # 256 channels per core
T = B * L                 # 2048 fused time steps
TPAD = T + B * (K - 1)    # padded x for causal conv: [3 zeros][b0][3 zeros][b1]
NT = T // 512             # moving-dim tiles of 512

_CACHE = {}


def _build_module():
    import concourse.bacc as bacc
    import concourse.bass as bass
    import concourse.mybir as mybir
    import concourse.tile as tile

    f32 = mybir.dt.float32
    f32r = mybir.dt.float32r
    bf16 = mybir.dt.bfloat16
    Alu = mybir.AluOpType
    Act = mybir.ActivationFunctionType

    nc = bacc.Bacc(
        "TRN2",
        target_bir_lowering=False,
        debug=False,
        num_devices=NCORES,
    )

    # ---- I/O -------------------------------------------------------------
    hsT = nc.dram_tensor("hsT", [H, T], f32, kind="ExternalInput").ap()
    winT = nc.dram_tensor("winT", [H, 2 * DP], f32, kind="ExternalInput").ap()
    wxT = nc.dram_tensor("wxT", [DP, R + 2 * N], f32, kind="ExternalInput").ap()
    wdtT = nc.dram_tensor("wdtT", [R, DP], f32, kind="ExternalInput").ap()
    bdt = nc.dram_tensor("bdt", [DP, 1], f32, kind="ExternalInput").ap()
    negA = nc.dram_tensor("negA", [DP, N], f32, kind="ExternalInput").ap()
    convw = nc.dram_tensor("convw", [DP, K], f32, kind="ExternalInput").ap()
    convb = nc.dram_tensor("convb", [DP, 1], f32, kind="ExternalInput").ap()
    dparam = nc.dram_tensor("dparam", [DP, 1], f32, kind="ExternalInput").ap()
    woutT = nc.dram_tensor("woutT", [DP, H], bf16, kind="ExternalInput").ap()
    ones_d = nc.dram_tensor("ones", [1, 128], f32, kind="ExternalInput").ap()
    outT = nc.dram_tensor("outT_part", [H, T], f32, kind="ExternalOutput").ap()

    DT2 = DP // 128  # d-tiles per core (2)

    with tile.TileContext(nc) as tc:
        with (
            tc.tile_pool(name="persist", bufs=1) as pp,
            tc.tile_pool(name="dram", bufs=1, space="DRAM") as dp,
        ):
            # ---------------- persistent SBUF tiles ----------------------
            xpad = [pp.tile([128, TPAD], f32, name=f"xpad{i}") for i in range(DT2)]
            x = [pp.tile([128, T], f32, name=f"x{i}") for i in range(DT2)]
            sg = [pp.tile([128, T], f32, name=f"sg{i}") for i in range(DT2)]
            dt_t = [pp.tile([128, T], f32, name=f"dt{i}") for i in range(DT2)]
            dtx = [pp.tile([128, T], f32, name=f"dtx{i}") for i in range(DT2)]
            y = [pp.tile([128, T], f32, name=f"y{i}") for i in range(DT2)]
            y_bf = [pp.tile([128, T], bf16, name=f"ybf{i}") for i in range(DT2)]
            dtlr_g = pp.tile([R, T], f32r, name="dtlr_g")

            convw_sb = [pp.tile([128, K], f32, name=f"convw_sb{i}") for i in range(DT2)]
            convb_sb = [pp.tile([128, 1], f32, name=f"convb_sb{i}") for i in range(DT2)]
            bdt_sb = [pp.tile([128, 1], f32, name=f"bdt_sb{i}") for i in range(DT2)]
            negA_sb = [pp.tile([128, N], f32, name=f"negA_sb{i}") for i in range(DT2)]
            dparam_sb = [
                pp.tile([128, 1], f32, name=f"dparam_sb{i}") for i in range(DT2)
            ]
            ones_sb = pp.tile([1, 128], f32r, name="ones_sb")
            wxT_sb = [
                pp.tile([128, R + 2 * N], f32, name=f"wxT_sb{i}") for i in range(DT2)
            ]
            wdtT_sb = pp.tile([R, DP], f32r, name="wdtT_sb")

            for i in range(DT2):
                rs = slice(128 * i, 128 * (i + 1))
                nc.sync.dma_start(convw_sb[i][:], convw[rs, :])
                nc.sync.dma_start(convb_sb[i][:], convb[rs, :])
                nc.sync.dma_start(bdt_sb[i][:], bdt[rs, :])
                nc.sync.dma_start(negA_sb[i][:], negA[rs, :])
                nc.sync.dma_start(dparam_sb[i][:], dparam[rs, :])
                nc.sync.dma_start(wxT_sb[i][:], wxT[rs, :])
            nc.sync.dma_start(wdtT_sb[:], wdtT.bitcast(f32r))
            nc.sync.dma_start(ones_sb[:], ones_d.bitcast(f32r))
            for i in range(DT2):
                nc.gpsimd.memset(xpad[i][:, 0:3], 0.0)
                nc.gpsimd.memset(xpad[i][:, 1027:1030], 0.0)

            # ---------------- phase 1: in_proj ----------------------------
            KH = H // 128  # 8 contraction tiles
            with (
                tc.tile_pool(name="ph1", bufs=1) as p1,
                tc.tile_pool(name="ps1", bufs=4, space="PSUM") as psm,
            ):
                hsT_sb = [p1.tile([128, T], f32r, name=f"hsT{k}") for k in range(KH)]
                winT_sb = [
                    p1.tile([128, 2 * DP], f32r, name=f"winT{k}") for k in range(KH)
                ]
                for k in range(KH):
                    nc.sync.dma_start(hsT_sb[k][:], hsT[128 * k : 128 * (k + 1), :].bitcast(f32r))
                    nc.sync.dma_start(winT_sb[k][:], winT[128 * k : 128 * (k + 1), :].bitcast(f32r))

                # m-tiles 0..DT2-1 -> x (pre-conv), DT2..2*DT2-1 -> gate
                for m in range(2 * DT2):
                    for t in range(NT):
                        pj = psm.tile([128, 512], f32, name="pj", tag="pj", bufs=4)
                        for k in range(KH):
                            nc.tensor.matmul(
                                pj[:],
                                winT_sb[k][:, 128 * m : 128 * (m + 1)],
                                hsT_sb[k][:, 512 * t : 512 * (t + 1)],
                                start=(k == 0),
                                stop=(k == KH - 1),
                            )
                        if m < DT2:
                            # pre-conv x -> padded layout (3-col zero pad per batch)
                            dst = 3 + 512 * t if t < 2 else 1030 + 512 * (t - 2)
                            nc.vector.tensor_copy(
                                xpad[m][:, dst : dst + 512], pj[:]
                            )
                        else:
                            nc.scalar.activation(
                                sg[m - DT2][:, 512 * t : 512 * (t + 1)],
                                pj[:],
                                Act.Silu,
                            )

            # ---------------- phase 2: depthwise causal conv --------------
            # xconv[d,t] = sum_k w[d,k] * xpad[d, t+k] per 1024-batch block
            for i in range(DT2):
                cw = convw_sb[i]
                for b in range(B):
                    base = (1024 + 3) * b
                    for k in range(K):
                        src = xpad[i][:, base + k : base + k + 1024]
                        dst = x[i][:, 1024 * b : 1024 * (b + 1)]
                        if k == 0:
                            nc.vector.tensor_scalar(
                                dst, src, cw[:, 0:1], None, Alu.mult
                            )
                        else:
                            nc.vector.scalar_tensor_tensor(
                                dst, src, cw[:, k : k + 1], dst, Alu.mult, Alu.add
                            )
                # x = silu(xconv + conv_b)
                nc.scalar.activation(
                    x[i][:], x[i][:], Act.Silu, bias=convb_sb[i][:]
                )

            # ---------------- phase 3: x_proj partial + AllReduce ---------
            ps3_cm = tc.tile_pool(name="ps3", bufs=1, space="PSUM")
            ps3 = ps3_cm.__enter__()
            sp_ps = ps3.tile([96, T], f32, name="sp_ps", tag="sp", bufs=1)
            for t in range(NT):
                for kd in range(DT2):
                    nc.tensor.matmul(
                        sp_ps[:, 512 * t : 512 * (t + 1)],
                        wxT_sb[kd][:],
                        x[kd][:, 512 * t : 512 * (t + 1)],
                        start=(kd == 0),
                        stop=(kd == DT2 - 1),
                    )
            ssm_local = pp.tile([96, T], f32, name="ssm_local")
            nc.vector.tensor_copy(ssm_local[:], sp_ps[:])

            ar_in = dp.tile([96, T], f32, name="ar_in")
            ar_out = dp.tile([96, T], f32, name="ar_out", addr_space="Shared")
            nc.sync.dma_start(ar_in[:], ssm_local[:])
            nc.gpsimd.collective_compute(
                "AllReduce",
                Alu.add,
                replica_groups=[list(range(NCORES))],
                ins=[ar_in[:]],
                outs=[ar_out[:]],
            )
            nc.sync.dma_start(dtlr_g[:], ar_out[0:R, :].bitcast(f32r))

            # ---------------- phase 4: dt = softplus(W_dt @ dt_lr + b) ----
            for m in range(DT2):
                dt_ps = ps3.tile([128, T], f32, name="dt_ps", tag="dtps", bufs=1)
                for t in range(NT):
                    nc.tensor.matmul(
                        dt_ps[:, 512 * t : 512 * (t + 1)],
                        wdtT_sb[:, 128 * m : 128 * (m + 1)],
                        dtlr_g[:, 512 * t : 512 * (t + 1)],
                        start=True,
                        stop=True,
                    )
                # softplus(z) = ln(exp(z) + 1); keeps ACT in the ln+exp table
                # (no table has Softplus; Exp here also serves the dA ops below)
                nc.scalar.activation(
                    dt_t[m][:],
                    dt_ps[:],
                    Act.Exp,
                    bias=bdt_sb[m][:],
                )
                nc.scalar.activation(dt_t[m][:], dt_t[m][:], Act.Ln, bias=1.0)
                nc.vector.tensor_mul(dtx[m][:], dt_t[m][:], x[m][:])

            # ---------------- phase 5: selective scan over 16 states ------
            ps3_cm.__exit__(None, None, None)
            with (
                tc.tile_pool(name="loop", bufs=2) as lp,
                tc.tile_pool(name="psB", bufs=1, space="PSUM") as psb,
            ):
                for n in range(N):
                    brow = lp.tile([1, T], f32r, name="brow", tag="brow", bufs=1)
                    crow = lp.tile([1, T], f32r, name="crow", tag="crow", bufs=1)
                    nc.sync.dma_start(brow[:], ar_out[R + n : R + n + 1, :].bitcast(f32r))
                    nc.sync.dma_start(crow[:], ar_out[R + N + n : R + N + n + 1, :].bitcast(f32r))
                    Bb = psb.tile([128, T], f32, name="Bb", tag="Bb")
                    Cb = psb.tile([128, T], f32, name="Cb", tag="Cb")
                    for t in range(NT):
                        nc.tensor.matmul(
                            Bb[:, 512 * t : 512 * (t + 1)],
                            ones_sb[:],
                            brow[:, 512 * t : 512 * (t + 1)],
                            start=True,
                            stop=True,
                        )
                        nc.tensor.matmul(
                            Cb[:, 512 * t : 512 * (t + 1)],
                            ones_sb[:],
                            crow[:, 512 * t : 512 * (t + 1)],
                            start=True,
                            stop=True,
                        )
                    for i in range(DT2):
                        dA = lp.tile([128, T], f32, name="dA", tag="dA")
                        dBu = lp.tile([128, T], f32, name="dBu", tag="dBu")
                        h = lp.tile([128, T], f32, name="h", tag="h", bufs=1)
                        g = lp.tile([128, T], f32, name="g", tag="g", bufs=1)
                        nc.scalar.activation(
                            dA[:],
                            dt_t[i][:],
                            Act.Exp,
                            scale=negA_sb[i][:, n : n + 1],
                        )
                        nc.vector.tensor_mul(dBu[:], dtx[i][:], Bb[:])
                        for b in range(B):
                            sl = slice(1024 * b, 1024 * (b + 1))
                            nc.vector.tensor_tensor_scan(
                                h[:, sl],
                                dA[:, sl],
                                dBu[:, sl],
                                0.0,
                                Alu.mult,
                                Alu.add,
                            )
                        if n == 0:
                            nc.vector.tensor_mul(y[i][:], h[:], Cb[:])
                        else:
                            nc.vector.tensor_mul(g[:], h[:], Cb[:])
                            nc.vector.tensor_add(y[i][:], y[i][:], g[:])

            # ---------------- phase 6: gate + out_proj --------------------
            woutT_sb = [
                pp.tile([128, H], bf16, name=f"woutT_sb{i}") for i in range(DT2)
            ]
            for i in range(DT2):
                nc.sync.dma_start(woutT_sb[i][:], woutT[128 * i : 128 * (i + 1), :])
            for i in range(DT2):
                # y = (y + x * D) * silu(gate)
                nc.vector.scalar_tensor_tensor(
                    y[i][:],
                    x[i][:],
                    dparam_sb[i][:],
                    y[i][:],
                    Alu.mult,
                    Alu.add,
                )
                nc.vector.tensor_mul(y_bf[i][:], y[i][:], sg[i][:])

            with (
                tc.tile_pool(name="ph6", bufs=4) as p6,
                tc.tile_pool(name="ps6", bufs=4, space="PSUM") as ps6,
            ):
                for m in range(H // 128):
                    for t in range(NT):
                        po = ps6.tile([128, 512], f32, name="po", tag="po", bufs=4)
                        for kd in range(DT2):
                            nc.tensor.matmul(
                                po[:],
                                woutT_sb[kd][
                                    :, 128 * m : 128 * (m + 1)
                                ],
                                y_bf[kd][:, 512 * t : 512 * (t + 1)],
                                start=(kd == 0),
                                stop=(kd == DT2 - 1),
                            )
                        ot = p6.tile([128, 512], f32, name="ot", tag="ot")
                        nc.vector.tensor_copy(ot[:], po[:])
                        nc.sync.dma_start(
                            outT[128 * m : 128 * (m + 1), 512 * t : 512 * (t + 1)],
                            ot[:],
                        )

    nc.compile()
    return nc


def _get_module():
    if "nc" not in _CACHE:
        _CACHE["nc"] = _build_module()
    return _CACHE["nc"]


def _shard_inputs(inputs):
    """Build the 8 per-core input maps (host-side transposes are free)."""
    hs = np.asarray(inputs["hidden_states"], dtype=np.float32)
    W_in = np.asarray(inputs["W_in"], dtype=np.float32)
    conv_w = np.asarray(inputs["conv_w"], dtype=np.float32)
    conv_b = np.asarray(inputs["conv_b"], dtype=np.float32)
    W_x = np.asarray(inputs["W_x"], dtype=np.float32)
    W_dt = np.asarray(inputs["W_dt"], dtype=np.float32)
    b_dt = np.asarray(inputs["b_dt"], dtype=np.float32)
    A_log = np.asarray(inputs["A_log"], dtype=np.float32)
    D_param = np.asarray(inputs["D_param"], dtype=np.float32)
    W_out = np.asarray(inputs["W_out"], dtype=np.float32)

    hsT = np.ascontiguousarray(hs.reshape(T, H).T)
    in_maps = []
    for c in range(NCORES):
        dc = slice(DP * c, DP * (c + 1))
        winT = np.ascontiguousarray(
            np.concatenate([W_in[dc], W_in[D + DP * c : D + DP * (c + 1)]], axis=0).T
        )
        in_maps.append(
            {
                "hsT": hsT,
                "ones": np.ones((1, 128), dtype=np.float32),
                "winT": winT,
                "wxT": np.ascontiguousarray(W_x[:, dc].T),
                "wdtT": np.ascontiguousarray(W_dt[dc].T),
                "bdt": np.ascontiguousarray(b_dt[dc][:, None]),
                "negA": np.ascontiguousarray(-np.exp(A_log[dc])),
                "convw": np.ascontiguousarray(conv_w[dc, 0, :]),
                "convb": np.ascontiguousarray(conv_b[dc][:, None]),
                "dparam": np.ascontiguousarray(D_param[dc][:, None]),
                "woutT": np.ascontiguousarray(W_out[:, dc].T).astype(
                    ml_dtypes.bfloat16
                ),
            }
        )
    return in_maps


def kernel(**inputs):
    from concourse import bass_utils

    nc = _get_module()
    in_maps = _shard_inputs(inputs)
    res = bass_utils.run_bass_kernel_spmd(
        nc, in_maps, core_ids=list(range(NCORES))
    )
    _CACHE["last_results"] = res
    acc = np.zeros((H, T), dtype=np.float32)
    for r in res.results:
        acc += r["outT_part"]
    return np.ascontiguousarray(acc.T).reshape(B, L, H)


# revision 13
# speedup vs baseline: 1.1522x; 1.1522x over previous
"""Trainium2 Bass kernel for EnhancedMambaMixer (B=2, L=1024, H=1024, D=2048, N=16, K=4, R=64).

Sharding: 8-way tensor-parallel over intermediate_size D (256 channels/core).
Each core computes its D-shard of in_proj/conv/scan and a partial out_proj;
a 786KB in-kernel AllReduce combines the x_proj partials (dt_lr/B/C are
reductions over the full D). Host sums the 8 out_proj partials.

Layout on-chip: channels in partitions, time fused as B*L=2048 in the free dim.

Engine plan (v2):
  PE    - in_proj (f32r), x_proj/dt matmuls, y = sum_n g_n via identity-matmul
          PSUM accumulation (bf16), out_proj (bf16)
  ACT   - SiLU, softplus (exp+ln, one act-table switch), the 32 exp(dt*-A_n),
          PSUM evictions
  DVE   - conv taps (stt), scans (2cyc/elem, DVE-only), g = h*C (bf16 2x)
  GPSIMD- dBu = dtx*B (bf16)
  DMA   - B/C row broadcasts replicated from DRAM (bf16)
"""

import ml_dtypes
import numpy as np

# Problem constants (hardcoded; kernel.py must be self-contained).
B, L, H = 2, 1024, 1024
D = 2048
N = 16
K = 4
R = 64
NCORES = 8
DP = D // NCORES          # 256 channels per core
T = B * L                 # 2048 fused time steps
TPAD = T + B * (K - 1)    # padded x for causal conv: [3 zeros][b0][3 zeros][b1]
NT = T // 512             # moving-dim tiles of 512

_CACHE = {}


def _build_module(dbu_on_gpsimd=True):
    import concourse.bacc as bacc
    import concourse.mybir as mybir
    import concourse.tile as tile

    f32 = mybir.dt.float32
    f32r = mybir.dt.float32r
    bf16 = mybir.dt.bfloat16
    Alu = mybir.AluOpType
    Act = mybir.ActivationFunctionType

    nc = bacc.Bacc(
        "TRN2",
        target_bir_lowering=False,
        debug=False,
        num_devices=NCORES,
    )

    # ---- I/O -------------------------------------------------------------
    hsT = nc.dram_tensor("hsT", [H, T], f32, kind="ExternalInput").ap()
    winT = nc.dram_tensor("winT", [H, 2 * DP], f32, kind="ExternalInput").ap()
    wxT = nc.dram_tensor("wxT", [DP, R + 2 * N], f32, kind="ExternalInput").ap()
    wdtT = nc.dram_tensor("wdtT", [R, DP], f32, kind="ExternalInput").ap()
    bdt = nc.dram_tensor("bdt", [DP, 1], f32, kind="ExternalInput").ap()
    negA = nc.dram_tensor("negA", [DP, N], f32, kind="ExternalInput").ap()
    convw = nc.dram_tensor("convw", [DP, K], f32, kind="ExternalInput").ap()
    convb = nc.dram_tensor("convb", [DP, 1], f32, kind="ExternalInput").ap()
    dparam = nc.dram_tensor("dparam", [DP, 1], f32, kind="ExternalInput").ap()
    woutT = nc.dram_tensor("woutT", [DP, H], bf16, kind="ExternalInput").ap()
    eye_d = nc.dram_tensor("eye", [128, 128], bf16, kind="ExternalInput").ap()
    outT = nc.dram_tensor("outT_part", [H, T], f32, kind="ExternalOutput").ap()

    DT2 = DP // 128  # d-tiles per core (2)

    with tile.TileContext(nc) as tc:
        with (
            tc.tile_pool(name="persist", bufs=1) as pp,
            tc.tile_pool(name="dram", bufs=1, space="DRAM") as dp,
        ):
            # ---------------- persistent SBUF tiles ----------------------
            xpad = [pp.tile([128, TPAD], f32, name=f"xpad{i}") for i in range(DT2)]
            x = [pp.tile([128, T], f32, name=f"x{i}") for i in range(DT2)]
            sg = [pp.tile([128, T], bf16, name=f"sg{i}") for i in range(DT2)]
            dt_t = [pp.tile([128, T], f32, name=f"dt{i}") for i in range(DT2)]
            dtx = [pp.tile([128, T], bf16, name=f"dtx{i}") for i in range(DT2)]
            yf = [pp.tile([128, T], bf16, name=f"yf{i}") for i in range(DT2)]
            dtlr_g = pp.tile([R, T], f32r, name="dtlr_g")
            eye_sb = pp.tile([128, 128], bf16, name="eye_sb")

            convw_sb = [pp.tile([128, K], f32, name=f"convw_sb{i}") for i in range(DT2)]
            convb_sb = [pp.tile([128, 1], f32, name=f"convb_sb{i}") for i in range(DT2)]
            bdt_sb = [pp.tile([128, 1], f32, name=f"bdt_sb{i}") for i in range(DT2)]
            negA_sb = [pp.tile([128, N], f32, name=f"negA_sb{i}") for i in range(DT2)]
            dparam_sb = [
                pp.tile([128, 1], f32, name=f"dparam_sb{i}") for i in range(DT2)
            ]
            wxT_sb = [
                pp.tile([128, R + 2 * N], f32, name=f"wxT_sb{i}") for i in range(DT2)
            ]
            wdtT_sb = pp.tile([R, DP], f32r, name="wdtT_sb")
            woutT_sb = [
                pp.tile([128, H], bf16, name=f"woutT_sb{i}") for i in range(DT2)
            ]

            for i in range(DT2):
                rs = slice(128 * i, 128 * (i + 1))
                nc.sync.dma_start(convw_sb[i][:], convw[rs, :])
                nc.sync.dma_start(convb_sb[i][:], convb[rs, :])
                nc.sync.dma_start(bdt_sb[i][:], bdt[rs, :])
                nc.sync.dma_start(negA_sb[i][:], negA[rs, :])
                nc.sync.dma_start(dparam_sb[i][:], dparam[rs, :])
                nc.sync.dma_start(wxT_sb[i][:], wxT[rs, :])
                nc.sync.dma_start(woutT_sb[i][:], woutT[rs, :])
            nc.sync.dma_start(wdtT_sb[:], wdtT.bitcast(f32r))
            nc.sync.dma_start(eye_sb[:], eye_d)
            for i in range(DT2):
                nc.gpsimd.memset(xpad[i][:, 0:3], 0.0)
                nc.gpsimd.memset(xpad[i][:, 1027:1030], 0.0)

            # ---------------- phase 1: in_proj ----------------------------
            KH = H // 128  # 8 contraction tiles
            with (
                tc.tile_pool(name="ph1", bufs=1) as p1,
                tc.tile_pool(name="ps1", bufs=4, space="PSUM") as ps1,
            ):
                hsT_sb = [p1.tile([128, T], f32r, name=f"hsT{k}") for k in range(KH)]
                winT_sb = [
                    p1.tile([128, 2 * DP], f32r, name=f"winT{k}") for k in range(KH)
                ]
                for k in range(KH):
                    nc.sync.dma_start(
                        hsT_sb[k][:], hsT[128 * k : 128 * (k + 1), :].bitcast(f32r)
                    )
                    nc.sync.dma_start(
                        winT_sb[k][:], winT[128 * k : 128 * (k + 1), :].bitcast(f32r)
                    )

                # m-tiles 0..DT2-1 -> x (pre-conv), DT2..2*DT2-1 -> gate
                for m in range(2 * DT2):
                    for t in range(NT):
                        pj = ps1.tile([128, 512], f32, name="pj", tag="pj", bufs=4)
                        for k in range(KH):
                            nc.tensor.matmul(
                                pj[:],
                                winT_sb[k][:, 128 * m : 128 * (m + 1)],
                                hsT_sb[k][:, 512 * t : 512 * (t + 1)],
                                start=(k == 0),
                                stop=(k == KH - 1),
                            )
                        if m < DT2:
                            # pre-conv x -> padded layout (3-col zero pad per batch)
                            dst = 3 + 512 * t if t < 2 else 1030 + 512 * (t - 2)
                            nc.scalar.copy(xpad[m][:, dst : dst + 512], pj[:])
                        else:
                            nc.scalar.activation(
                                sg[m - DT2][:, 512 * t : 512 * (t + 1)],
                                pj[:],
                                Act.Silu,
                            )

            # ---------------- phase 2: depthwise causal conv --------------
            # xconv[d,t] = sum_k w[d,k] * xpad[d, t+k] per 1024-batch block
            for i in range(DT2):
                cw = convw_sb[i]
                for b in range(B):
                    base = (1024 + 3) * b
                    for k in range(K):
                        src = xpad[i][:, base + k : base + k + 1024]
                        dst = x[i][:, 1024 * b : 1024 * (b + 1)]
                        if k == 0:
                            nc.vector.tensor_scalar(
                                dst, src, cw[:, 0:1], None, Alu.mult
                            )
                        else:
                            nc.vector.scalar_tensor_tensor(
                                dst, src, cw[:, k : k + 1], dst, Alu.mult, Alu.add
                            )
                # x = silu(xconv + conv_b)
                nc.scalar.activation(x[i][:], x[i][:], Act.Silu, bias=convb_sb[i][:])

            # ---------------- phase 3: x_proj partial + AllReduce ---------
            ps3_cm = tc.tile_pool(name="ps3", bufs=1, space="PSUM")
            ps3 = ps3_cm.__enter__()
            sp_ps = ps3.tile([96, T], f32, name="sp_ps", tag="sp", bufs=1)
            for t in range(NT):
                for kd in range(DT2):
                    nc.tensor.matmul(
                        sp_ps[:, 512 * t : 512 * (t + 1)],
                        wxT_sb[kd][:],
                        x[kd][:, 512 * t : 512 * (t + 1)],
                        start=(kd == 0),
                        stop=(kd == DT2 - 1),
                    )
            ssm_local = pp.tile([96, T], f32, name="ssm_local")
            nc.vector.tensor_copy(ssm_local[:], sp_ps[:])

            ar_in = dp.tile([96, T], f32, name="ar_in")
            ar_out = dp.tile([96, T], f32, name="ar_out", addr_space="Shared")
            nc.sync.dma_start(ar_in[:], ssm_local[:])
            nc.gpsimd.collective_compute(
                "AllReduce",
                Alu.add,
                replica_groups=[list(range(NCORES))],
                ins=[ar_in[:]],
                outs=[ar_out[:]],
            )
            nc.sync.dma_start(dtlr_g[:], ar_out[0:R, :].bitcast(f32r))

            # B/C rows -> bf16 -> DRAM, for replicating broadcast DMAs
            bc_sb = pp.tile([2 * N, T], f32, name="bc_sb")
            bc_bf = pp.tile([2 * N, T], bf16, name="bc_bf")
            bc_dram = dp.tile([2 * N, T], bf16, name="bc_dram")
            nc.sync.dma_start(bc_sb[:], ar_out[R : R + 2 * N, :])
            nc.vector.tensor_copy(bc_bf[:], bc_sb[:])
            nc.sync.dma_start(bc_dram[:], bc_bf[:])

            # ---------------- phase 4: dt = softplus(W_dt @ dt_lr + b) ----
            for m in range(DT2):
                dt_ps = ps3.tile([128, T], f32, name="dt_ps", tag="dtps", bufs=1)
                for t in range(NT):
                    nc.tensor.matmul(
                        dt_ps[:, 512 * t : 512 * (t + 1)],
                        wdtT_sb[:, 128 * m : 128 * (m + 1)],
                        dtlr_g[:, 512 * t : 512 * (t + 1)],
                        start=True,
                        stop=True,
                    )
                # softplus(z) = ln(exp(z) + 1); keeps ACT in the ln+exp table
                # (no table has Softplus; Exp here also serves the dA ops below)
                nc.scalar.activation(
                    dt_t[m][:],
                    dt_ps[:],
                    Act.Exp,
                    bias=bdt_sb[m][:],
                )
                nc.scalar.activation(dt_t[m][:], dt_t[m][:], Act.Ln, bias=1.0)
                nc.vector.tensor_mul(dtx[m][:], dt_t[m][:], x[m][:])
            ps3_cm.__exit__(None, None, None)

            # ---------------- phase 5: selective scan over 16 states ------
            with (
                tc.tile_pool(name="loop", bufs=2) as lp,
                tc.tile_pool(name="psY", bufs=1, space="PSUM") as psy,
            ):
                y_ps = [
                    psy.tile([128, T], f32, name=f"y_ps{i}", tag=f"y{i}")
                    for i in range(DT2)
                ]
                dbu_engine = nc.gpsimd if dbu_on_gpsimd else nc.vector
                for n in range(N):
                    Bb = lp.tile([128, T], bf16, name="Bb", tag="Bb", bufs=3)
                    Cb = lp.tile([128, T], bf16, name="Cb", tag="Cb", bufs=3)
                    nc.sync.dma_start(
                        Bb[:], bc_dram[n : n + 1, :].to_broadcast([128, T])
                    )
                    nc.sync.dma_start(
                        Cb[:], bc_dram[N + n : N + n + 1, :].to_broadcast([128, T])
                    )
                    for i in range(DT2):
                        dA = lp.tile([128, T], f32, name="dA", tag="dA")
                        dBu = lp.tile([128, T], bf16, name="dBu", tag="dBu")
                        h = lp.tile([128, T], bf16, name="h", tag="h")
                        g = lp.tile([128, T], bf16, name="g", tag="g")
                        nc.scalar.activation(
                            dA[:],
                            dt_t[i][:],
                            Act.Exp,
                            scale=negA_sb[i][:, n : n + 1],
                        )
                        dbu_engine.tensor_tensor(
                            out=dBu[:], in0=dtx[i][:], in1=Bb[:], op=Alu.mult
                        )
                        for b in range(B):
                            sl = slice(1024 * b, 1024 * (b + 1))
                            nc.vector.tensor_tensor_scan(
                                h[:, sl],
                                dA[:, sl],
                                dBu[:, sl],
                                0.0,
                                Alu.mult,
                                Alu.add,
                            )
                        nc.vector.tensor_mul(g[:], h[:], Cb[:])
                        for t in range(NT):
                            nc.tensor.matmul(
                                y_ps[i][:, 512 * t : 512 * (t + 1)],
                                eye_sb[:],
                                g[:, 512 * t : 512 * (t + 1)],
                                start=(n == 0),
                                stop=(n == N - 1),
                            )

                # ---- gate: yf = (y + x*D) * silu(gate), bf16 -------------
                for i in range(DT2):
                    tmp = lp.tile([128, T], bf16, name="tmp", tag="tmp")
                    nc.vector.scalar_tensor_tensor(
                        tmp[:],
                        x[i][:],
                        dparam_sb[i][:],
                        y_ps[i][:],
                        Alu.mult,
                        Alu.add,
                    )
                    nc.vector.tensor_mul(yf[i][:], tmp[:], sg[i][:])

            # ---------------- phase 6: out_proj ---------------------------
            with (
                tc.tile_pool(name="ph6", bufs=4) as p6,
                tc.tile_pool(name="ps6", bufs=4, space="PSUM") as ps6,
            ):
                for m in range(H // 128):
                    for t in range(NT):
                        po = ps6.tile([128, 512], f32, name="po", tag="po", bufs=4)
                        for kd in range(DT2):
                            nc.tensor.matmul(
                                po[:],
                                woutT_sb[kd][:, 128 * m : 128 * (m + 1)],
                                yf[kd][:, 512 * t : 512 * (t + 1)],
                                start=(kd == 0),
                                stop=(kd == DT2 - 1),
                            )
                        ot = p6.tile([128, 512], f32, name="ot", tag="ot")
                        nc.scalar.copy(ot[:], po[:])
                        nc.sync.dma_start(
                            outT[128 * m : 128 * (m + 1), 512 * t : 512 * (t + 1)],
                            ot[:],
                        )

    nc.compile()
    return nc


def _get_module():
    if "nc" not in _CACHE:
        _CACHE["nc"] = _build_module()
    return _CACHE["nc"]


def _shard_inputs(inputs):
    """Build the 8 per-core input maps (host-side transposes are free)."""
    hs = np.asarray(inputs["hidden_states"], dtype=np.float32)
    W_in = np.asarray(inputs["W_in"], dtype=np.float32)
    conv_w = np.asarray(inputs["conv_w"], dtype=np.float32)
    conv_b = np.asarray(inputs["conv_b"], dtype=np.float32)
    W_x = np.asarray(inputs["W_x"], dtype=np.float32)
    W_dt = np.asarray(inputs["W_dt"], dtype=np.float32)
    b_dt = np.asarray(inputs["b_dt"], dtype=np.float32)
    A_log = np.asarray(inputs["A_log"], dtype=np.float32)
    D_param = np.asarray(inputs["D_param"], dtype=np.float32)
    W_out = np.asarray(inputs["W_out"], dtype=np.float32)

    hsT = np.ascontiguousarray(hs.reshape(T, H).T)
    in_maps = []
    for c in range(NCORES):
        dc = slice(DP * c, DP * (c + 1))
        winT = np.ascontiguousarray(
            np.concatenate([W_in[dc], W_in[D + DP * c : D + DP * (c + 1)]], axis=0).T
        )
        in_maps.append(
            {
                "hsT": hsT,
                "eye": np.eye(128, dtype=np.float32).astype(ml_dtypes.bfloat16),
                "winT": winT,
                "wxT": np.ascontiguousarray(W_x[:, dc].T),
                "wdtT": np.ascontiguousarray(W_dt[dc].T),
                "bdt": np.ascontiguousarray(b_dt[dc][:, None]),
                "negA": np.ascontiguousarray(-np.exp(A_log[dc])),
                "convw": np.ascontiguousarray(conv_w[dc, 0, :]),
                "convb": np.ascontiguousarray(conv_b[dc][:, None]),
                "dparam": np.ascontiguousarray(D_param[dc][:, None]),
                "woutT": np.ascontiguousarray(W_out[:, dc].T).astype(
                    ml_dtypes.bfloat16
                ),
            }
        )
    return in_maps


def kernel(**inputs):
    from concourse import bass_utils

    nc = _get_module()
    in_maps = _shard_inputs(inputs)
    res = bass_utils.run_bass_kernel_spmd(
        nc, in_maps, core_ids=list(range(NCORES))
    )
    _CACHE["last_results"] = res
    acc = np.zeros((H, T), dtype=np.float32)
    for r in res.results:
        acc += r["outT_part"]
    return np.ascontiguousarray(acc.T).reshape(B, L, H)


# revision 14
# speedup vs baseline: 1.2891x; 1.1189x over previous
"""Trainium2 Bass kernel for EnhancedMambaMixer (B=2, L=1024, H=1024, D=2048, N=16, K=4, R=64).

Sharding: 8-way tensor-parallel over intermediate_size D (256 channels/core).
Each core computes its D-shard of in_proj/conv/scan and a partial out_proj;
a 786KB in-kernel AllReduce combines the x_proj partials (dt_lr/B/C are
reductions over the full D). Host sums the 8 out_proj partials.

Layout on-chip: channels in partitions, time fused as B*L=2048 in the free dim.

Engine plan (v2):
  PE    - in_proj (f32r), x_proj/dt matmuls, y = sum_n g_n via identity-matmul
          PSUM accumulation (bf16), out_proj (bf16)
  ACT   - SiLU, softplus (exp+ln, one act-table switch), the 32 exp(dt*-A_n),
          PSUM evictions
  DVE   - conv taps (stt), scans (2cyc/elem, DVE-only), g = h*C (bf16 2x)
  GPSIMD- dBu = dtx*B (bf16)
  DMA   - B/C row broadcasts replicated from DRAM (bf16)
"""

import ml_dtypes
import numpy as np

# Problem constants (hardcoded; kernel.py must be self-contained).
B, L, H = 2, 1024, 1024
D = 2048
N = 16
K = 4
R = 64
NCORES = 8
DP = D // NCORES          # 256 channels per core
T = B * L                 # 2048 fused time steps
TPAD = T + B * (K - 1)    # padded x for causal conv: [3 zeros][b0][3 zeros][b1]
NT = T // 512             # moving-dim tiles of 512

_CACHE = {}


def _build_module(dbu_on_gpsimd=False):
    import concourse.bacc as bacc
    import concourse.mybir as mybir
    import concourse.tile as tile

    f32 = mybir.dt.float32
    f32r = mybir.dt.float32r
    bf16 = mybir.dt.bfloat16
    Alu = mybir.AluOpType
    Act = mybir.ActivationFunctionType

    nc = bacc.Bacc(
        "TRN2",
        target_bir_lowering=False,
        debug=False,
        num_devices=NCORES,
    )

    # ---- I/O -------------------------------------------------------------
    hsT = nc.dram_tensor("hsT", [H, T], f32, kind="ExternalInput").ap()
    winT = nc.dram_tensor("winT", [H, 2 * DP], f32, kind="ExternalInput").ap()
    wxT = nc.dram_tensor("wxT", [DP, R + 2 * N], f32, kind="ExternalInput").ap()
    wdtT = nc.dram_tensor("wdtT", [R, DP], f32, kind="ExternalInput").ap()
    bdt = nc.dram_tensor("bdt", [DP, 1], f32, kind="ExternalInput").ap()
    negA = nc.dram_tensor("negA", [DP, N], f32, kind="ExternalInput").ap()
    convw = nc.dram_tensor("convw", [DP, K], f32, kind="ExternalInput").ap()
    convb = nc.dram_tensor("convb", [DP, 1], f32, kind="ExternalInput").ap()
    dparam = nc.dram_tensor("dparam", [DP, 1], f32, kind="ExternalInput").ap()
    woutT = nc.dram_tensor("woutT", [DP, H], bf16, kind="ExternalInput").ap()
    eye_d = nc.dram_tensor("eye", [128, 128], bf16, kind="ExternalInput").ap()
    outT = nc.dram_tensor("outT_part", [H, T], f32, kind="ExternalOutput").ap()

    DT2 = DP // 128  # d-tiles per core (2)

    with tile.TileContext(nc) as tc:
        with (
            tc.tile_pool(name="persist", bufs=1) as pp,
            tc.tile_pool(name="dram", bufs=1, space="DRAM") as dp,
        ):
            # ---------------- persistent SBUF tiles ----------------------
            xpad = [pp.tile([128, TPAD], f32, name=f"xpad{i}") for i in range(DT2)]
            x = [pp.tile([128, T], f32, name=f"x{i}") for i in range(DT2)]
            sg = [pp.tile([128, T], bf16, name=f"sg{i}") for i in range(DT2)]
            dt_t = [pp.tile([128, T], f32, name=f"dt{i}") for i in range(DT2)]
            dtx = [pp.tile([128, T], bf16, name=f"dtx{i}") for i in range(DT2)]
            yf = [pp.tile([128, T], bf16, name=f"yf{i}") for i in range(DT2)]
            dtlr_g = pp.tile([R, T], f32r, name="dtlr_g")
            eye_sb = pp.tile([128, 128], bf16, name="eye_sb")

            convw_sb = [pp.tile([128, K], f32, name=f"convw_sb{i}") for i in range(DT2)]
            convb_sb = [pp.tile([128, 1], f32, name=f"convb_sb{i}") for i in range(DT2)]
            bdt_sb = [pp.tile([128, 1], f32, name=f"bdt_sb{i}") for i in range(DT2)]
            negA_sb = [pp.tile([128, N], f32, name=f"negA_sb{i}") for i in range(DT2)]
            dparam_sb = [
                pp.tile([128, 1], f32, name=f"dparam_sb{i}") for i in range(DT2)
            ]
            wxT_sb = [
                pp.tile([128, R + 2 * N], f32, name=f"wxT_sb{i}") for i in range(DT2)
            ]
            wdtT_sb = pp.tile([R, DP], f32r, name="wdtT_sb")
            woutT_sb = [
                pp.tile([128, H], bf16, name=f"woutT_sb{i}") for i in range(DT2)
            ]

            for i in range(DT2):
                rs = slice(128 * i, 128 * (i + 1))
                nc.sync.dma_start(convw_sb[i][:], convw[rs, :])
                nc.sync.dma_start(convb_sb[i][:], convb[rs, :])
                nc.sync.dma_start(bdt_sb[i][:], bdt[rs, :])
                nc.sync.dma_start(negA_sb[i][:], negA[rs, :])
                nc.sync.dma_start(dparam_sb[i][:], dparam[rs, :])
                nc.sync.dma_start(wxT_sb[i][:], wxT[rs, :])
                nc.sync.dma_start(woutT_sb[i][:], woutT[rs, :])
            nc.sync.dma_start(wdtT_sb[:], wdtT.bitcast(f32r))
            nc.sync.dma_start(eye_sb[:], eye_d)
            for i in range(DT2):
                nc.gpsimd.memset(xpad[i][:, 0:3], 0.0)
                nc.gpsimd.memset(xpad[i][:, 1027:1030], 0.0)

            # ---------------- phase 1: in_proj ----------------------------
            KH = H // 128  # 8 contraction tiles
            with (
                tc.tile_pool(name="ph1", bufs=1) as p1,
                tc.tile_pool(name="ps1", bufs=4, space="PSUM") as ps1,
            ):
                hsT_sb = [p1.tile([128, T], f32r, name=f"hsT{k}") for k in range(KH)]
                winT_sb = [
                    p1.tile([128, 2 * DP], f32r, name=f"winT{k}") for k in range(KH)
                ]
                for k in range(KH):
                    nc.sync.dma_start(
                        hsT_sb[k][:], hsT[128 * k : 128 * (k + 1), :].bitcast(f32r)
                    )
                    nc.sync.dma_start(
                        winT_sb[k][:], winT[128 * k : 128 * (k + 1), :].bitcast(f32r)
                    )

                # m-tiles 0..DT2-1 -> x (pre-conv), DT2..2*DT2-1 -> gate
                for m in range(2 * DT2):
                    for t in range(NT):
                        pj = ps1.tile([128, 512], f32, name="pj", tag="pj", bufs=4)
                        for k in range(KH):
                            nc.tensor.matmul(
                                pj[:],
                                winT_sb[k][:, 128 * m : 128 * (m + 1)],
                                hsT_sb[k][:, 512 * t : 512 * (t + 1)],
                                start=(k == 0),
                                stop=(k == KH - 1),
                            )
                        if m < DT2:
                            # pre-conv x -> padded layout (3-col zero pad per batch)
                            dst = 3 + 512 * t if t < 2 else 1030 + 512 * (t - 2)
                            nc.scalar.copy(xpad[m][:, dst : dst + 512], pj[:])
                        else:
                            nc.scalar.activation(
                                sg[m - DT2][:, 512 * t : 512 * (t + 1)],
                                pj[:],
                                Act.Silu,
                            )

            # ---------------- phase 2: depthwise causal conv --------------
            # xconv[d,t] = sum_k w[d,k] * xpad[d, t+k] per 1024-batch block
            for i in range(DT2):
                cw = convw_sb[i]
                for b in range(B):
                    base = (1024 + 3) * b
                    for k in range(K):
                        src = xpad[i][:, base + k : base + k + 1024]
                        dst = x[i][:, 1024 * b : 1024 * (b + 1)]
                        if k == 0:
                            nc.vector.tensor_scalar(
                                dst, src, cw[:, 0:1], None, Alu.mult
                            )
                        else:
                            nc.vector.scalar_tensor_tensor(
                                dst, src, cw[:, k : k + 1], dst, Alu.mult, Alu.add
                            )
                # x = silu(xconv + conv_b)
                nc.scalar.activation(x[i][:], x[i][:], Act.Silu, bias=convb_sb[i][:])

            # ---------------- phase 3: x_proj partial + AllReduce ---------
            ps3_cm = tc.tile_pool(name="ps3", bufs=1, space="PSUM")
            ps3 = ps3_cm.__enter__()
            sp_ps = ps3.tile([96, T], f32, name="sp_ps", tag="sp", bufs=1)
            for t in range(NT):
                for kd in range(DT2):
                    nc.tensor.matmul(
                        sp_ps[:, 512 * t : 512 * (t + 1)],
                        wxT_sb[kd][:],
                        x[kd][:, 512 * t : 512 * (t + 1)],
                        start=(kd == 0),
                        stop=(kd == DT2 - 1),
                    )
            ssm_local = pp.tile([96, T], f32, name="ssm_local")
            nc.vector.tensor_copy(ssm_local[:], sp_ps[:])

            ar_in = dp.tile([96, T], f32, name="ar_in")
            ar_out = dp.tile([96, T], f32, name="ar_out", addr_space="Shared")
            nc.sync.dma_start(ar_in[:], ssm_local[:])
            nc.gpsimd.collective_compute(
                "AllReduce",
                Alu.add,
                replica_groups=[list(range(NCORES))],
                ins=[ar_in[:]],
                outs=[ar_out[:]],
            )
            nc.sync.dma_start(dtlr_g[:], ar_out[0:R, :].bitcast(f32r))

            # B/C rows -> bf16 -> DRAM, for replicating broadcast DMAs
            bc_sb = pp.tile([2 * N, T], f32, name="bc_sb")
            bc_bf = pp.tile([2 * N, T], bf16, name="bc_bf")
            bc_dram = dp.tile([2 * N, T], bf16, name="bc_dram")
            nc.sync.dma_start(bc_sb[:], ar_out[R : R + 2 * N, :])
            nc.vector.tensor_copy(bc_bf[:], bc_sb[:])
            nc.sync.dma_start(bc_dram[:], bc_bf[:])

            # ---------------- phase 4: dt = softplus(W_dt @ dt_lr + b) ----
            for m in range(DT2):
                dt_ps = ps3.tile([128, T], f32, name="dt_ps", tag="dtps", bufs=1)
                for t in range(NT):
                    nc.tensor.matmul(
                        dt_ps[:, 512 * t : 512 * (t + 1)],
                        wdtT_sb[:, 128 * m : 128 * (m + 1)],
                        dtlr_g[:, 512 * t : 512 * (t + 1)],
                        start=True,
                        stop=True,
                    )
                # softplus(z) = ln(exp(z) + 1); keeps ACT in the ln+exp table
                # (no table has Softplus; Exp here also serves the dA ops below)
                nc.scalar.activation(
                    dt_t[m][:],
                    dt_ps[:],
                    Act.Exp,
                    bias=bdt_sb[m][:],
                )
                nc.scalar.activation(dt_t[m][:], dt_t[m][:], Act.Ln, bias=1.0)
                nc.vector.tensor_mul(dtx[m][:], dt_t[m][:], x[m][:])
            ps3_cm.__exit__(None, None, None)

            # ---------------- phase 5: selective scan over 16 states ------
            with (
                tc.tile_pool(name="loop", bufs=2) as lp,
                tc.tile_pool(name="psY", bufs=1, space="PSUM") as psy,
            ):
                y_ps = [
                    psy.tile([128, T], f32, name=f"y_ps{i}", tag=f"y{i}")
                    for i in range(DT2)
                ]
                dbu_engine = nc.gpsimd if dbu_on_gpsimd else nc.vector
                for n in range(N):
                    Bb = lp.tile([128, T], bf16, name="Bb", tag="Bb", bufs=3)
                    Cb = lp.tile([128, T], bf16, name="Cb", tag="Cb", bufs=3)
                    nc.sync.dma_start(
                        Bb[:], bc_dram[n : n + 1, :].to_broadcast([128, T])
                    )
                    nc.sync.dma_start(
                        Cb[:], bc_dram[N + n : N + n + 1, :].to_broadcast([128, T])
                    )
                    for i in range(DT2):
                        dA = lp.tile([128, T], f32, name="dA", tag="dA")
                        dBu = lp.tile([128, T], bf16, name="dBu", tag="dBu")
                        h = lp.tile([128, T], bf16, name="h", tag="h")
                        g = lp.tile([128, T], bf16, name="g", tag="g")
                        nc.scalar.activation(
                            dA[:],
                            dt_t[i][:],
                            Act.Exp,
                            scale=negA_sb[i][:, n : n + 1],
                        )
                        dbu_engine.tensor_tensor(
                            out=dBu[:], in0=dtx[i][:], in1=Bb[:], op=Alu.mult
                        )
                        for b in range(B):
                            sl = slice(1024 * b, 1024 * (b + 1))
                            nc.vector.tensor_tensor_scan(
                                h[:, sl],
                                dA[:, sl],
                                dBu[:, sl],
                                0.0,
                                Alu.mult,
                                Alu.add,
                            )
                        nc.vector.tensor_mul(g[:], h[:], Cb[:])
                        for t in range(NT):
                            nc.tensor.matmul(
                                y_ps[i][:, 512 * t : 512 * (t + 1)],
                                eye_sb[:],
                                g[:, 512 * t : 512 * (t + 1)],
                                start=(n == 0),
                                stop=(n == N - 1),
                            )

                # ---- gate: yf = (y + x*D) * silu(gate), bf16 -------------
                for i in range(DT2):
                    tmp = lp.tile([128, T], bf16, name="tmp", tag="tmp")
                    nc.vector.scalar_tensor_tensor(
                        tmp[:],
                        x[i][:],
                        dparam_sb[i][:],
                        y_ps[i][:],
                        Alu.mult,
                        Alu.add,
                    )
                    nc.vector.tensor_mul(yf[i][:], tmp[:], sg[i][:])

            # ---------------- phase 6: out_proj ---------------------------
            with (
                tc.tile_pool(name="ph6", bufs=4) as p6,
                tc.tile_pool(name="ps6", bufs=4, space="PSUM") as ps6,
            ):
                for m in range(H // 128):
                    for t in range(NT):
                        po = ps6.tile([128, 512], f32, name="po", tag="po", bufs=4)
                        for kd in range(DT2):
                            nc.tensor.matmul(
                                po[:],
                                woutT_sb[kd][:, 128 * m : 128 * (m + 1)],
                                yf[kd][:, 512 * t : 512 * (t + 1)],
                                start=(kd == 0),
                                stop=(kd == DT2 - 1),
                            )
                        ot = p6.tile([128, 512], f32, name="ot", tag="ot")
                        nc.scalar.copy(ot[:], po[:])
                        nc.sync.dma_start(
                            outT[128 * m : 128 * (m + 1), 512 * t : 512 * (t + 1)],
                            ot[:],
                        )

    nc.compile()
    return nc


def _get_module():
    if "nc" not in _CACHE:
        _CACHE["nc"] = _build_module()
    return _CACHE["nc"]


def _shard_inputs(inputs):
    """Build the 8 per-core input maps (host-side transposes are free)."""
    hs = np.asarray(inputs["hidden_states"], dtype=np.float32)
    W_in = np.asarray(inputs["W_in"], dtype=np.float32)
    conv_w = np.asarray(inputs["conv_w"], dtype=np.float32)
    conv_b = np.asarray(inputs["conv_b"], dtype=np.float32)
    W_x = np.asarray(inputs["W_x"], dtype=np.float32)
    W_dt = np.asarray(inputs["W_dt"], dtype=np.float32)
    b_dt = np.asarray(inputs["b_dt"], dtype=np.float32)
    A_log = np.asarray(inputs["A_log"], dtype=np.float32)
    D_param = np.asarray(inputs["D_param"], dtype=np.float32)
    W_out = np.asarray(inputs["W_out"], dtype=np.float32)

    hsT = np.ascontiguousarray(hs.reshape(T, H).T)
    in_maps = []
    for c in range(NCORES):
        dc = slice(DP * c, DP * (c + 1))
        winT = np.ascontiguousarray(
            np.concatenate([W_in[dc], W_in[D + DP * c : D + DP * (c + 1)]], axis=0).T
        )
        in_maps.append(
            {
                "hsT": hsT,
                "eye": np.eye(128, dtype=np.float32).astype(ml_dtypes.bfloat16),
                "winT": winT,
                "wxT": np.ascontiguousarray(W_x[:, dc].T),
                "wdtT": np.ascontiguousarray(W_dt[dc].T),
                "bdt": np.ascontiguousarray(b_dt[dc][:, None]),
                "negA": np.ascontiguousarray(-np.exp(A_log[dc])),
                "convw": np.ascontiguousarray(conv_w[dc, 0, :]),
                "convb": np.ascontiguousarray(conv_b[dc][:, None]),
                "dparam": np.ascontiguousarray(D_param[dc][:, None]),
                "woutT": np.ascontiguousarray(W_out[:, dc].T).astype(
                    ml_dtypes.bfloat16
                ),
            }
        )
    return in_maps


def kernel(**inputs):
    from concourse import bass_utils

    nc = _get_module()
    in_maps = _shard_inputs(inputs)
    res = bass_utils.run_bass_kernel_spmd(
        nc, in_maps, core_ids=list(range(NCORES))
    )
    _CACHE["last_results"] = res
    acc = np.zeros((H, T), dtype=np.float32)
    for r in res.results:
        acc += r["outT_part"]
    return np.ascontiguousarray(acc.T).reshape(B, L, H)
